# revision 1
# baseline (speedup 1.0000x reference)
"""Two-layer GCN (PyG GCNConv semantics) on 8 Trainium2 NeuronCores.

Math: out = Ahat @ relu(Ahat @ (X@W1) + b1) @ W2 + b2, with
Ahat = D^-1/2 (A + I) D^-1/2.  The edge normalization factors as
dinv[src]*dinv[dst], so per layer we:
  - pre-scale the source table rows by dinv (folded into PSUM eviction),
  - aggregate with a 0/1-times-dinv_dst one-hot matrix per 128-edge tile
    (segmented matmul on the PE, edges sorted by dst),
  - gather source rows from HBM by index via gpsimd dma_gather (int16
    indices, so the 50000-row table is addressed as two 25000-row halves).

Sharding: destination nodes are split across the 8 cores (6250 each).
Layer-1 dense matmul X@W1 is replicated on every core (cheaper than an
all-gather of the table).  One AllGather shares the layer-2 source table.
"""

import sys

import numpy as np

try:
    import concourse.bass as bass  # noqa: F401
except ImportError:
    sys.path.insert(0, "/opt/trn_rl_repo")

from contextlib import ExitStack

import ml_dtypes

import concourse.bass as bass
import concourse.tile as tile
from concourse import bacc, mybir
from concourse.bass_utils import run_bass_kernel_spmd

BF16 = ml_dtypes.bfloat16

# debug ablation: 0 = no dma_gather + no collective, 1 = gather + no collective,
# 2 = full kernel
ABLATE = 2

N = 50000
E = 800000
FIN = 128
HID = 128
FOUT = 64
NCORES = 8
NSH = N // NCORES  # 6250 destination nodes per core
BLK = 128  # dst block (psum window)
NBLK = (NSH + BLK - 1) // BLK  # 49
SBW = 4  # dst blocks per superblock (one 512-wide psum bank)
NSB = (NBLK + SBW - 1) // SBW  # 13
HALF = 25000  # table half split (int16 gather indices)
NPAD = ((N + 127) // 128) * 128  # 50048
NDTILES = NPAD // 128  # 391


def _layout(tiles):
    """Static program layout from per-(block,half) tile counts.

    Returns (TT, tile_base[NBLK][2], seg: {(sb,h): (tile0, ntiles)}).
    Data/program order: for sb, for half, for block in sb, k tiles.
    """
    tile_base = np.zeros((NBLK, 2), dtype=np.int64)
    seg = {}
    pos = 0
    for sb in range(NSB):
        blocks = range(sb * SBW, min((sb + 1) * SBW, NBLK))
        for h in (0, 1):
            seg_start = pos
            for b in blocks:
                tile_base[b][h] = pos
                pos += int(tiles[b][h])
            seg[(sb, h)] = (seg_start, pos - seg_start)
    return int(pos), tile_base, seg


def _prep(edge_index):
    src = np.asarray(edge_index[0], dtype=np.int64)
    dst = np.asarray(edge_index[1], dtype=np.int64)
    deg = (np.bincount(dst, minlength=N) + 1).astype(np.float64)
    dinv = (1.0 / np.sqrt(deg)).astype(np.float32)

    s_all = np.concatenate([src, np.arange(N, dtype=np.int64)])
    d_all = np.concatenate([dst, np.arange(N, dtype=np.int64)])
    core = d_all // NSH
    local = d_all % NSH
    block = local // BLK
    sbk = block // SBW
    half = (s_all >= HALF).astype(np.int64)

    cidx = (core * NBLK + block) * 2 + half
    cnt = np.bincount(cidx, minlength=NCORES * NBLK * 2).reshape(NCORES, NBLK, 2)
    tiles = ((cnt + BLK - 1) // BLK).max(axis=0)  # [NBLK, 2] max over cores

    TT, tile_base, seg = _layout(tiles)
    S = TT * BLK

    # sort edges into (core, sb, half, block) segment order
    order = np.lexsort((local, block, half, sbk, core))
    s_s = s_all[order]
    d_s = d_all[order]
    core_s = core[order]
    block_s = block[order]
    half_s = half[order]

    gid = (core_s * NBLK + block_s) * 2 + half_s
    change = np.r_[True, gid[1:] != gid[:-1]]
    gstart = np.maximum.accumulate(np.where(change, np.arange(len(gid)), 0))
    rank = np.arange(len(gid)) - gstart
    slot = tile_base[block_s, half_s] * BLK + rank  # per-core slot in [0, S)

    src_loc = np.where(half_s == 0, s_s, s_s - HALF).astype(np.int16)
    dst_loc = (d_s % NSH - block_s * BLK).astype(np.float32)  # 0..127
    dinv_d = dinv[d_s] * dinv[s_s]  # full edge norm dinv_src*dinv_dst

    seg_slot0 = np.zeros((NSB, 2), dtype=np.int64)
    for (sb, h), (t0, _nt) in seg.items():
        seg_slot0[sb, h] = t0 * BLK

    meta_np = np.zeros((NCORES, 128, TT, 2), dtype=np.float32)
    idx_np = np.zeros((NCORES, 128, S // 16), dtype=np.int16)
    for c in range(NCORES):
        m = core_s == c
        sl = slot[m]
        tt = sl // BLK
        pp = sl % BLK
        meta_np[c, pp, tt, 0] = dst_loc[m]
        meta_np[c, pp, tt, 1] = dinv_d[m]
        seg0 = seg_slot0[block_s[m] // SBW, half_s[m]]
        j = sl - seg0
        col = seg0 // 16 + j // 16
        row = j % 16
        v = src_loc[m]
        for g in range(8):  # replicate across the 8 gpsimd 16-partition groups
            idx_np[c, row + 16 * g, col] = v

    dinv_blk = np.zeros((NCORES, 128, NBLK), dtype=np.float32)
    ids = np.arange(NBLK * 128)
    valid = ids < NSH
    for c in range(NCORES):
        tmp = np.zeros(NBLK * 128, np.float32)
        tmp[valid] = dinv[c * NSH + ids[valid]]
        dinv_blk[c] = tmp.reshape(NBLK, 128).T

    tmp2 = np.zeros(NPAD, np.float32)
    tmp2[:N] = dinv
    dinv_dense = np.ascontiguousarray(tmp2.reshape(NDTILES, 128).T)  # [128, NDTILES]

    return tiles, dinv, meta_np, idx_np, dinv_blk, dinv_dense


def _build(tiles):
    """Build the (single, SPMD) Bacc program for the given tile counts."""
    TT, tile_base, seg = _layout(tiles)
    S = TT * BLK
    f32 = mybir.dt.float32
    bf16 = mybir.dt.bfloat16
    i16 = mybir.dt.int16
    AF = mybir.ActivationFunctionType
    OP = mybir.AluOpType

    nc = bacc.Bacc("TRN2", target_bir_lowering=False, debug=False, num_devices=NCORES)
    xT = nc.dram_tensor("xT", [128, NPAD], bf16, kind="ExternalInput")
    w1 = nc.dram_tensor("w1", [128, HID], bf16, kind="ExternalInput")
    w2 = nc.dram_tensor("w2", [128, FOUT], bf16, kind="ExternalInput")
    b1r = nc.dram_tensor("b1r", [128, HID], f32, kind="ExternalInput")
    b2r = nc.dram_tensor("b2r", [128, FOUT], f32, kind="ExternalInput")
    iot = nc.dram_tensor("iot", [128, BLK], bf16, kind="ExternalInput")
    meta = nc.dram_tensor("meta", [128, TT, 2], f32, kind="ExternalInput")
    idxt = nc.dram_tensor("idx", [128, S // 16], i16, kind="ExternalInput")
    outp = nc.dram_tensor("out", [NSH, FOUT], f32, kind="ExternalOutput")

    with tile.TileContext(nc) as tc, ExitStack() as ctx:
        const = ctx.enter_context(tc.tile_pool(name="const", bufs=1))
        dram = ctx.enter_context(tc.tile_pool(name="dram", bufs=1, space="DRAM"))
        xin = ctx.enter_context(tc.tile_pool(name="xin", bufs=4))
        t1ev = ctx.enter_context(tc.tile_pool(name="t1ev", bufs=4))
        gpool = ctx.enter_context(tc.tile_pool(name="g", bufs=3))
        tpp = ctx.enter_context(tc.tile_pool(name="tp", bufs=12))
        evp = ctx.enter_context(tc.tile_pool(name="ev", bufs=4))
        psd = ctx.enter_context(tc.tile_pool(name="psd", bufs=2, space="PSUM"))
        psa = ctx.enter_context(tc.tile_pool(name="psa", bufs=4, space="PSUM"))
        pso = ctx.enter_context(tc.tile_pool(name="pso", bufs=2, space="PSUM"))

        def cload(ap, shape, dtype, tag):
            t = const.tile(shape, dtype, tag=tag)
            nc.sync.dma_start(t[:], ap)
            return t

        w1_sb = cload(w1[:, :], [128, HID], bf16, "w1")
        w2_sb = cload(w2[:, :], [128, FOUT], bf16, "w2")
        b1_sb = cload(b1r[:, :], [128, HID], f32, "b1")
        b2_sb = cload(b2r[:, :], [128, FOUT], f32, "b2")
        iota_sb = cload(iot[:, :], [128, BLK], bf16, "iota")
        meta_sb = cload(meta[:, :, :], [128, TT, 2], f32, "meta")
        idx_sb = cload(idxt[:, :], [128, S // 16], i16, "idx")

        table1 = dram.tile([NPAD, HID], bf16, tag="table1")
        t2loc = dram.tile([NSH, HID], bf16, tag="t2loc")
        t2full = dram.tile([N, HID], bf16, tag="t2full")

        # Phase A: table1 = X @ W1, replicated on every core (norm lives in meta)
        DB = 4  # node tiles per DMA batch
        for j0 in range(0, NDTILES, DB):
            nb = min(DB, NDTILES - j0)
            xt = xin.tile([128, DB * 128], bf16, tag="xt")
            nc.sync.dma_start(
                xt[:, 0 : nb * 128], xT[:, j0 * 128 : (j0 + nb) * 128]
            )
            ev = t1ev.tile([128, DB, HID], bf16, tag="t1ev")
            for i in range(nb):
                ps = psd.tile([128, HID], f32, tag="psd")
                nc.tensor.matmul(
                    ps[:],
                    lhsT=xt[:, i * 128 : (i + 1) * 128],
                    rhs=w1_sb[:],
                    start=True,
                    stop=True,
                )
                nc.scalar.activation(ev[:, i, :], ps[:], AF.Copy)
            # table1 row j*128+p <- ev[p, j-j0, :]
            nc.sync.dma_start(
                table1[j0 * 128 : (j0 + nb) * 128, :].rearrange(
                    "(t p) f -> p t f", p=128
                ),
                ev[:, 0:nb, :],
            )

        def agg(layer):
            table = table1 if layer == 1 else t2full
            for sb in range(NSB):
                blocks = list(range(sb * SBW, min((sb + 1) * SBW, NBLK)))
                nbl = len(blocks)
                gt = {}
                for h in (0, 1):
                    t0, ntl = seg[(sb, h)]
                    if ntl == 0:
                        continue
                    g = gpool.tile([128, ntl, 128], bf16, tag=f"g{h}")
                    if ABLATE >= 1:
                        view = table[0:HALF, :] if h == 0 else table[HALF : 2 * HALF, :]
                        # SWDGE descriptor ring limit: <= 1024 idx per gather
                        GCH = 8  # tiles per gather chunk
                        for q0 in range(0, ntl, GCH):
                            qn = min(GCH, ntl - q0)
                            c0 = (t0 + q0) * 8  # idx columns (tile*128/16)
                            nc.gpsimd.dma_gather(
                                out_ap=g[:, q0 : q0 + qn, :],
                                in_ap=view,
                                idxs_ap=idx_sb[:, c0 : c0 + qn * 8],
                                num_idxs=qn * 128,
                                num_idxs_reg=qn * 128,
                                elem_size=HID,
                                queue_num=0,
                            )
                    else:
                        nc.vector.memset(g[:], 0)
                    gt[h] = g
                for b in blocks:
                    ps = psa.tile([128, BLK], f32, tag="psa")  # one bank per block
                    for h in (0, 1):
                        if seg[(sb, h)][1] == 0 or tiles[b][h] == 0:
                            continue
                        seg_t0 = seg[(sb, h)][0]
                        for k in range(int(tiles[b][h])):
                            t = int(tile_base[b][h]) + k
                            gofs = t - seg_t0
                            first = (k == 0) and (h == 0 or tiles[b][0] == 0)
                            last = (k == int(tiles[b][h]) - 1) and (
                                h == 1 or tiles[b][1] == 0
                            )
                            tp = tpp.tile([128, BLK], bf16, tag="tp")
                            nc.vector.tensor_scalar(
                                out=tp[:],
                                in0=iota_sb[:],
                                scalar1=meta_sb[:, t, 0:1],
                                scalar2=meta_sb[:, t, 1:2],
                                op0=OP.is_equal,
                                op1=OP.mult,
                            )
                            gtile = gt[h][:, gofs, :]
                            if layer == 1:
                                nc.tensor.matmul(
                                    ps[:], lhsT=tp[:], rhs=gtile, start=first, stop=last
                                )
                            else:
                                nc.tensor.matmul(
                                    ps[:], lhsT=gtile, rhs=tp[:], start=first, stop=last
                                )
                    r0 = b * BLK
                    r1 = min(NSH, r0 + BLK)
                    if layer == 1:
                        s1 = evp.tile([128, HID], f32, tag="s1")
                        nc.vector.tensor_add(s1[:], ps[:], b1_sb[:])
                        ev = evp.tile([128, HID], bf16, tag="t2ev")
                        nc.scalar.activation(ev[:], s1[:], AF.Relu)
                        nc.sync.dma_start(t2loc[r0:r1, :], ev[0 : r1 - r0, :])
                    else:
                        ag2 = evp.tile([128, BLK], bf16, tag="ag2")
                        nc.scalar.activation(ag2[:], ps[:], AF.Copy)
                        po = pso.tile([128, FOUT], f32, tag="pso")
                        nc.tensor.matmul(
                            po[:], lhsT=ag2[:], rhs=w2_sb[:], start=True, stop=True
                        )
                        oo = evp.tile([128, FOUT], f32, tag="oo")
                        nc.vector.tensor_add(oo[:], po[:], b2_sb[:])
                        nc.sync.dma_start(outp[r0:r1, :], oo[0 : r1 - r0, :])

        agg(1)
        if ABLATE >= 2:
            nc.gpsimd.collective_compute(
                "AllGather",
                mybir.AluOpType.bypass,
                replica_groups=[list(range(NCORES))],
                ins=[t2loc[:].opt()],
                outs=[t2full[:].opt()],
            )
        else:
            nc.sync.dma_start(t2full[0:NSH, :], t2loc[:, :])
        agg(2)

    nc.finalize()
    return nc


def _in_maps(x, W1, b1, W2, b2, prep):
    tiles, dinv, meta_np, idx_np, dinv_blk, dinv_dense = prep
    xT = np.zeros((128, NPAD), dtype=BF16)
    xT[:, :N] = np.asarray(x, np.float32).T.astype(BF16)
    w1b = np.asarray(W1, np.float32).astype(BF16)
    w2b = np.asarray(W2, np.float32).astype(BF16)
    b1rep = np.broadcast_to(np.asarray(b1, np.float32), (128, HID)).copy()
    b2rep = np.broadcast_to(np.asarray(b2, np.float32), (128, FOUT)).copy()
    iota = np.broadcast_to(np.arange(BLK, dtype=np.float32), (128, BLK)).astype(BF16)
    shared = {
        "xT": xT,
        "w1": w1b,
        "w2": w2b,
        "b1r": b1rep,
        "b2r": b2rep,
        "iot": np.ascontiguousarray(iota),
    }
    return [
        dict(
            shared,
            meta=np.ascontiguousarray(meta_np[c]),
            idx=np.ascontiguousarray(idx_np[c]),
        )
        for c in range(NCORES)
    ]


def kernel(x, edge_index, W1, b1, W2, b2):
    prep = _prep(edge_index)
    nc = _build(prep[0])
    in_maps = _in_maps(x, W1, b1, W2, b2, prep)
    res = run_bass_kernel_spmd(nc, in_maps, core_ids=list(range(NCORES)), trace=False)
    out = np.concatenate(
        [res.results[c]["out"].astype(np.float32) for c in range(NCORES)], axis=0
    )
    return out



# revision 44
# speedup vs baseline: 2.4623x; 2.4623x over previous
"""Two-layer GCN (PyG GCNConv semantics) on 8 Trainium2 NeuronCores.

Math: out = Ahat @ relu(Ahat @ (X@W1) + b1) @ W2 + b2, with
Ahat = D^-1/2 (A + I) D^-1/2.  Aggregation commutes with the dense
weight matmul (Ahat @ (X@W) = (Ahat @ X) @ W), so each layer:
  - gathers RAW feature rows (x for layer 1, h for layer 2) by source
    index via gpsimd dma_gather (bf16 256B rows; int16 indices, so each
    source table is addressed as two <=32767-row halves),
  - aggregates them with a 0/1-times-norm one-hot matrix per 128-edge
    tile (segmented matmul on the PE, edges sorted by dst; psum is
    [feat, dst]),
  - applies the layer weight matmul + bias (+relu) per 128-dst block.

Gathers are spread round-robin over 4 SWDGE queues (parallel Q7
descriptor generation — the dominant cost).

Sharding: destination nodes are split across the 8 cores (6250 each).
The layer-2 source table is shared via TWO AllGathers (rows 0:3125 and
3125:6250 of each core's h), the first issued halfway through layer 1.
Layer 2 runs two passes (h0 pass stashes per-block partials in SBUF,
h1 pass combines) so the second AllGather overlaps the h0 pass.
"""

import sys

import numpy as np

try:
    import concourse.bass as bass  # noqa: F401
except ImportError:
    sys.path.insert(0, "/opt/trn_rl_repo")

from contextlib import ExitStack

import ml_dtypes

import concourse.bass as bass
import concourse.tile as tile
from concourse import bacc, mybir
from concourse.bass_utils import run_bass_kernel_spmd

BF16 = ml_dtypes.bfloat16

# debug ablation: 0 = no dma_gather + no collective, 1 = gather + no collective,
# 2 = full kernel
ABLATE = 2

N = 50000
E = 800000
FIN = 128
HID = 128
FOUT = 64
NCORES = 8
NSH = N // NCORES  # 6250 destination nodes per core
BLK = 128  # dst block (psum window)
NBLK = (NSH + BLK - 1) // BLK  # 49
SBW = 4  # dst blocks per superblock
NSB = (NBLK + SBW - 1) // SBW  # 13
HALF = 25000  # layer-1 table half split (int16 gather indices)
HSH = NSH // 2  # 3125: layer-2 per-core half split
CC0_SB = 6  # issue the first AllGather after this layer-1 superblock
GCH = 8  # tiles per gather chunk (1024 idx: ucode max per dma_gather)
SCRATCH = 98304  # dynamic_dma_scratch_size (SWDGE descriptor ring space)
SHARED = True  # Shared addr_space for the AllGather outputs
CCMODE = 2  # 2 = two split AllGathers (first issued mid-layer-1), 1 = one
REPEAT = 1  # bench-only: run the whole computation this many times per launch
ONEHOT = 2  # bench-only: 2=full tensor_scalar, 1=1-col (cheap)
AGGMM = 2  # bench-only: 2=all aggregation matmuls, 1=one per (block,half)
EPI = 1  # bench-only: 1=full W-matmul epilogue, 0=direct eviction
GBUFS = 2  # gather pool buffers per half-tag
QUEUES = 4  # SWDGE queues; gathers round-robin across them (parallel desc-gen)
SPACKET = True  # dma_gather single_packet flag


def _layout(tiles):
    """Static program layout from per-(block,half) tile counts.

    Returns (TT, tile_base[NBLK][2], seg: {(sb,h): (tile0, ntiles)}).
    Data/program order: for sb, for half, for block in sb, k tiles.
    """
    tile_base = np.zeros((NBLK, 2), dtype=np.int64)
    seg = {}
    pos = 0
    for sb in range(NSB):
        blocks = range(sb * SBW, min((sb + 1) * SBW, NBLK))
        for h in (0, 1):
            seg_start = pos
            for b in blocks:
                tile_base[b][h] = pos
                pos += int(tiles[b][h])
            seg[(sb, h)] = (seg_start, pos - seg_start)
    return int(pos), tile_base, seg


def _prep_layer(core, local, block, sbk, half, src_row, dst_loc, dinv_e):
    """Per-layer edge layout: tile counts, one-hot meta, gather indices."""
    cidx = (core * NBLK + block) * 2 + half
    cnt = np.bincount(cidx, minlength=NCORES * NBLK * 2).reshape(NCORES, NBLK, 2)
    tiles = ((cnt + BLK - 1) // BLK).max(axis=0)  # [NBLK, 2] max over cores

    TT, tile_base, seg = _layout(tiles)
    S = TT * BLK

    order = np.lexsort((local, block, half, sbk, core))
    core_s = core[order]
    block_s = block[order]
    half_s = half[order]

    gid = (core_s * NBLK + block_s) * 2 + half_s
    change = np.r_[True, gid[1:] != gid[:-1]]
    gstart = np.maximum.accumulate(np.where(change, np.arange(len(gid)), 0))
    rank = np.arange(len(gid)) - gstart
    slot = tile_base[block_s, half_s] * BLK + rank  # per-core slot in [0, S)

    seg_slot0 = np.zeros((NSB, 2), dtype=np.int64)
    for (sb, h), (t0, _nt) in seg.items():
        seg_slot0[sb, h] = t0 * BLK

    meta_np = np.zeros((NCORES, 128, TT, 2), dtype=np.float32)
    idx_np = np.zeros((NCORES, 128, S // 16), dtype=np.int16)
    row_s = src_row[order]
    dst_s = dst_loc[order]
    din_s = dinv_e[order]
    for c in range(NCORES):
        m = core_s == c
        sl = slot[m]
        tt = sl // BLK
        pp = sl % BLK
        meta_np[c, pp, tt, 0] = dst_s[m]
        meta_np[c, pp, tt, 1] = din_s[m]
        seg0 = seg_slot0[block_s[m] // SBW, half_s[m]]
        j = sl - seg0
        col = seg0 // 16 + j // 16
        row = j % 16
        v = row_s[m]
        for g in range(8):  # replicate across the 8 gpsimd 16-partition groups
            idx_np[c, row + 16 * g, col] = v
    return tiles, meta_np, idx_np


def _prep(edge_index):
    """Returns (l1, l2) layer preps; l2 is for CCMODE=2 (split tables).
    CCMODE=1 reuses l1 for layer 2 (same half rule, node-order table)."""
    src = np.asarray(edge_index[0], dtype=np.int64)
    dst = np.asarray(edge_index[1], dtype=np.int64)
    deg = (np.bincount(dst, minlength=N) + 1).astype(np.float64)
    dinv = (1.0 / np.sqrt(deg)).astype(np.float32)

    s_all = np.concatenate([src, np.arange(N, dtype=np.int64)])
    d_all = np.concatenate([dst, np.arange(N, dtype=np.int64)])
    core = d_all // NSH
    local = d_all % NSH
    block = local // BLK
    sbk = block // SBW
    dst_loc = (local - block * BLK).astype(np.float32)  # 0..127
    dinv_e = dinv[d_all] * dinv[s_all]  # full edge norm dinv_src*dinv_dst

    # layer 1: table is x in node order, halves split at 25000
    half1 = (s_all >= HALF).astype(np.int64)
    row1 = (s_all - half1 * HALF).astype(np.int16)
    l1 = _prep_layer(core, local, block, sbk, half1, row1, dst_loc, dinv_e)

    # layer 2: tables t2A/t2B are [core][3125] row blocks
    sc = s_all // NSH
    r = s_all % NSH
    half2 = (r >= HSH).astype(np.int64)
    row2 = (sc * HSH + r - half2 * HSH).astype(np.int16)
    l2 = _prep_layer(core, local, block, sbk, half2, row2, dst_loc, dinv_e)

    return l1, l2, dinv


def _build(tiles1, tiles2):
    """Build the (single, SPMD) Bacc program for the given tile counts."""
    f32 = mybir.dt.float32
    bf16 = mybir.dt.bfloat16
    i16 = mybir.dt.int16
    AF = mybir.ActivationFunctionType
    OP = mybir.AluOpType
    if CCMODE == 1:
        tiles2 = tiles1
    lay = {1: _layout(tiles1), 2: _layout(tiles2)}  # TT, tile_base, seg
    tiles = {1: tiles1, 2: tiles2}

    nc = bacc.Bacc(
        "TRN2",
        target_bir_lowering=False,
        debug=False,
        num_devices=NCORES,
        dynamic_dma_scratch_size=SCRATCH,
        num_swdge_queues=QUEUES,
    )
    xr = nc.dram_tensor("xr", [N, FIN], bf16, kind="ExternalInput")
    w1 = nc.dram_tensor("w1", [128, HID], bf16, kind="ExternalInput")
    w2 = nc.dram_tensor("w2", [128, FOUT], bf16, kind="ExternalInput")
    b1r = nc.dram_tensor("b1r", [128, HID], f32, kind="ExternalInput")
    b2r = nc.dram_tensor("b2r", [128, FOUT], f32, kind="ExternalInput")
    iot = nc.dram_tensor("iot", [128, BLK], bf16, kind="ExternalInput")
    metas, idxts = {}, {}
    for L in (1, 2):
        TT = lay[L][0]
        metas[L] = nc.dram_tensor(f"meta{L}", [128, TT, 2], f32, kind="ExternalInput")
        idxts[L] = nc.dram_tensor(f"idx{L}", [128, TT * 8], i16, kind="ExternalInput")
    outp = nc.dram_tensor("out", [NSH, FOUT], f32, kind="ExternalOutput")

    with tile.TileContext(nc) as tc, ExitStack() as ctx:
        const = ctx.enter_context(tc.tile_pool(name="const", bufs=1))
        dram = ctx.enter_context(tc.tile_pool(name="dram", bufs=1, space="DRAM"))
        gpool = ctx.enter_context(tc.tile_pool(name="g", bufs=GBUFS))
        tpp = ctx.enter_context(tc.tile_pool(name="tp", bufs=4))
        evp = ctx.enter_context(tc.tile_pool(name="ev", bufs=4))
        stash = ctx.enter_context(tc.tile_pool(name="stash", bufs=NBLK))
        psa = ctx.enter_context(tc.tile_pool(name="psa", bufs=4, space="PSUM"))
        pso = ctx.enter_context(tc.tile_pool(name="pso", bufs=2, space="PSUM"))

        def cload(ap, shape, dtype, tag):
            t = const.tile(shape, dtype, tag=tag)
            nc.sync.dma_start(t[:], ap)
            return t

        w1_sb = cload(w1[:, :], [128, HID], bf16, "w1")
        w2_sb = cload(w2[:, :], [128, FOUT], bf16, "w2")
        b1_sb = cload(b1r[:, :], [128, HID], f32, "b1")
        b2_sb = cload(b2r[:, :], [128, FOUT], f32, "b2")
        iota_sb = cload(iot[:, :], [128, BLK], bf16, "iota")
        meta_sb, idx_sb = {}, {}
        for L in (1, 2):
            TT = lay[L][0]
            meta_sb[L] = cload(metas[L][:, :, :], [128, TT, 2], f32, f"meta{L}")
            idx_sb[L] = cload(idxts[L][:, :], [128, TT * 8], i16, f"idx{L}")

        qrr = [0]  # round-robin queue counter

        def gather_seg(L, sb, h, table_h):
            """Gather segment (sb, h) of layer L from its half-table."""
            TT, tile_base, seg = lay[L]
            t0, ntl = seg[(sb, h)]
            if ntl == 0:
                return None
            g = gpool.tile([128, ntl, 128], bf16, tag=f"g{h}")
            if ABLATE >= 1:
                for q0 in range(0, ntl, GCH):
                    qn = min(GCH, ntl - q0)
                    c0 = (t0 + q0) * 8  # idx columns (tile*128/16)
                    nc.gpsimd.dma_gather(
                        out_ap=g[:, q0 : q0 + qn, :],
                        in_ap=table_h,
                        idxs_ap=idx_sb[L][:, c0 : c0 + qn * 8],
                        num_idxs=qn * 128,
                        num_idxs_reg=qn * 128,
                        elem_size=128,
                        queue_num=qrr[0] % QUEUES,
                        single_packet=SPACKET,
                    )
                    qrr[0] += 1
            else:
                nc.vector.memset(g[:], 0)
            return g

        def acc_tiles(L, b, h, g, ps, first_of_ps, last_of_ps):
            """Accumulate block b's half-h tiles into psum window ps.

            One pair of broadcast DVE ops builds ALL nt one-hot tiles:
            tp[p,t,j] = (iota[j] == dst[p,t]) * norm[p,t].
            """
            TT, tile_base, seg = lay[L]
            nt = int(tiles[L][b][h])
            seg_t0 = seg[(b // SBW, h)][0]
            t0 = int(tile_base[b][h])
            tpw = tpp.tile([128, nt, BLK], bf16, tag="tp")
            bn = nt if ONEHOT == 2 else 1
            iota_b = iota_sb[:].rearrange("p (o f) -> p o f", o=1).broadcast_to(
                [128, bn, BLK]
            )
            nc.vector.tensor_tensor(
                out=tpw[:, 0:bn, :],
                in0=iota_b,
                in1=meta_sb[L][:, t0 : t0 + bn, 0:1].broadcast_to([128, bn, BLK]),
                op=OP.is_equal,
            )
            nc.vector.tensor_tensor(
                out=tpw[:, 0:bn, :],
                in0=tpw[:, 0:bn, :],
                in1=meta_sb[L][:, t0 : t0 + bn, 1:2].broadcast_to([128, bn, BLK]),
                op=OP.mult,
            )
            for k in range(nt):
                gofs = t0 + k - seg_t0
                if AGGMM == 2 or (first_of_ps and k == 0):
                    nc.tensor.matmul(
                        ps[:],
                        lhsT=g[:, gofs, :],
                        rhs=tpw[:, k, :],
                        start=first_of_ps and k == 0,
                        stop=(last_of_ps and k == nt - 1) if AGGMM == 2 else True,
                    )

        for it in range(REPEAT):
            t2loc = dram.tile([NSH, HID], bf16, tag=f"t2loc{it}")
            addr = "Shared" if SHARED else "Local"
            if CCMODE == 2:
                t2A_t = dram.tile(
                    [NCORES * HSH, HID], bf16, tag=f"t2A{it}", addr_space=addr
                )
                t2B_t = dram.tile(
                    [NCORES * HSH, HID], bf16, tag=f"t2B{it}", addr_space=addr
                )
                t2A, t2B = t2A_t[:, :], t2B_t[:, :]
            else:
                t2full = dram.tile([N, HID], bf16, tag=f"t2f{it}", addr_space=addr)
                t2A = t2full[0:HALF, :]
                t2B = t2full[HALF : 2 * HALF, :]

            def issue_cc(half):
                if CCMODE == 1:
                    if half == 0:
                        return  # single collective, issued at end of layer 1
                    ins, outs, nrows = t2loc[:, :], t2full[:, :], NSH
                else:
                    ins = t2loc[0:HSH, :] if half == 0 else t2loc[HSH:NSH, :]
                    outs = (t2A_t if half == 0 else t2B_t)[:, :]
                    nrows = HSH
                if ABLATE >= 2:
                    nc.gpsimd.collective_compute(
                        "AllGather",
                        mybir.AluOpType.bypass,
                        replica_groups=[list(range(NCORES))],
                        ins=[ins.opt()],
                        outs=[outs.opt()],
                    )
                else:
                    nc.sync.dma_start(outs.tensor[0:nrows, :], ins)

            # ---- layer 1: interleaved halves, one psum window per block ----
            for sb in range(NSB):
                blocks = list(range(sb * SBW, min((sb + 1) * SBW, NBLK)))
                gt = {}
                for h in (0, 1):
                    table_h = xr[0:HALF, :] if h == 0 else xr[HALF : 2 * HALF, :]
                    gt[h] = gather_seg(1, sb, h, table_h)
                for b in blocks:
                    ps = psa.tile([128, BLK], f32, tag="psa")  # [feat, dst]
                    n0, n1 = int(tiles1[b][0]), int(tiles1[b][1])
                    for h in (0, 1):
                        if (n0 if h == 0 else n1) == 0 or gt[h] is None:
                            continue
                        acc_tiles(
                            1, b, h, gt[h], ps,
                            first_of_ps=(h == 0 or n0 == 0),
                            last_of_ps=(h == 1 or n1 == 0),
                        )
                    r0 = b * BLK
                    r1 = min(NSH, r0 + BLK)
                    ag = evp.tile([128, BLK], bf16, tag="ag")
                    nc.scalar.activation(ag[:], ps[:], AF.Copy)
                    if EPI:
                        po = pso.tile([128, HID], f32, tag="pso")
                        nc.tensor.matmul(
                            po[:], lhsT=ag[:], rhs=w1_sb[:], start=True, stop=True
                        )
                        s1 = evp.tile([128, HID], f32, tag="s1")
                        nc.vector.tensor_add(s1[:], po[:], b1_sb[:])
                        ev = evp.tile([128, HID], bf16, tag="t2ev")
                        nc.scalar.activation(ev[:], s1[:], AF.Relu)
                        nc.sync.dma_start(t2loc[r0:r1, :], ev[0 : r1 - r0, :])
                    else:
                        nc.sync.dma_start(t2loc[r0:r1, :], ag[0 : r1 - r0, :])
                if sb == CC0_SB:
                    issue_cc(0)
            issue_cc(1)

            # ---- layer 2, pass A: halves h0 from t2A; stash partials ----
            agA = [None] * NBLK
            for sb in range(NSB):
                blocks = list(range(sb * SBW, min((sb + 1) * SBW, NBLK)))
                g = gather_seg(2, sb, 0, t2A)
                for b in blocks:
                    if int(tiles2[b][0]) == 0 or g is None:
                        continue
                    ps = psa.tile([128, BLK], f32, tag="psa")
                    acc_tiles(2, b, 0, g, ps, first_of_ps=True, last_of_ps=True)
                    a = stash.tile([128, BLK], bf16, tag="agA")
                    nc.scalar.activation(a[:], ps[:], AF.Copy)
                    agA[b] = a

            # ---- layer 2, pass B: halves h1 from t2B; combine + epilogue ----
            for sb in range(NSB):
                blocks = list(range(sb * SBW, min((sb + 1) * SBW, NBLK)))
                g = gather_seg(2, sb, 1, t2B)
                for b in blocks:
                    n1 = int(tiles2[b][1])
                    lhs = None
                    if n1 > 0 and g is not None:
                        ps = psa.tile([128, BLK], f32, tag="psa")
                        acc_tiles(2, b, 1, g, ps, first_of_ps=True, last_of_ps=True)
                        agB = evp.tile([128, BLK], bf16, tag="agB")
                        if agA[b] is not None:
                            nc.vector.tensor_tensor(
                                out=agB[:], in0=ps[:], in1=agA[b][:], op=OP.add
                            )
                        else:
                            nc.scalar.activation(agB[:], ps[:], AF.Copy)
                        lhs = agB
                    else:
                        lhs = agA[b]
                    r0 = b * BLK
                    r1 = min(NSH, r0 + BLK)
                    oo = evp.tile([128, FOUT], f32, tag="oo")
                    if EPI:
                        po = pso.tile([128, FOUT], f32, tag="pso2")
                        nc.tensor.matmul(
                            po[:], lhsT=lhs[:], rhs=w2_sb[:], start=True, stop=True
                        )
                        nc.vector.tensor_add(oo[:], po[:], b2_sb[:])
                    else:
                        nc.scalar.activation(oo[:], lhs[:, 0:FOUT], AF.Copy)
                    nc.sync.dma_start(outp[r0:r1, :], oo[0 : r1 - r0, :])

    nc.finalize()
    return nc


def _in_maps(x, W1, b1, W2, b2, prep):
    (tiles1, meta1, idx1), (tiles2, meta2, idx2) = prep[0], prep[1]
    xrb = np.asarray(x, np.float32).astype(BF16)
    w1b = np.asarray(W1, np.float32).astype(BF16)
    w2b = np.asarray(W2, np.float32).astype(BF16)
    b1rep = np.broadcast_to(np.asarray(b1, np.float32), (128, HID)).copy()
    b2rep = np.broadcast_to(np.asarray(b2, np.float32), (128, FOUT)).copy()
    iota = np.broadcast_to(np.arange(BLK, dtype=np.float32), (128, BLK)).astype(BF16)
    shared = {
        "xr": xrb,
        "w1": w1b,
        "w2": w2b,
        "b1r": b1rep,
        "b2r": b2rep,
        "iot": np.ascontiguousarray(iota),
    }
    if CCMODE == 1:
        meta2, idx2 = meta1, idx1
    return [
        dict(
            shared,
            meta1=np.ascontiguousarray(meta1[c]),
            idx1=np.ascontiguousarray(idx1[c]),
            meta2=np.ascontiguousarray(meta2[c]),
            idx2=np.ascontiguousarray(idx2[c]),
        )
        for c in range(NCORES)
    ]


def kernel(x, edge_index, W1, b1, W2, b2):
    prep = _prep(edge_index)
    nc = _build(prep[0][0], prep[1][0])
    in_maps = _in_maps(x, W1, b1, W2, b2, prep)
    res = run_bass_kernel_spmd(nc, in_maps, core_ids=list(range(NCORES)), trace=False)
    out = np.concatenate(
        [res.results[c]["out"].astype(np.float32) for c in range(NCORES)], axis=0
    )
    return out
